# revision 1
# baseline (speedup 1.0000x reference)
"""Masked dot-product attention (B=64, L=1024, D=64) on 8 NeuronCores — v2.

Data-parallel over batch (8 slots/core, batches rank-sorted so one
instruction stream fits all cores; per-slot key-block counts baked at build).

Key design vs v1 (62976ns):
  - All inputs bf16 (error budget allows: rel ~5e-3 from bf16 alone).
  - S^T[k,q] per 128-key block via 2x bf16 matmuls (512 moving cols each).
  - exp is split across TWO engine lanes so the Activation engine is no
    longer the bottleneck:
      * ACT lane: exact exp activation (scale=1/8, per-partition mask bias),
        bf16 out. (1024+222)cyc @1.2GHz = 1038ns/block.
      * DVE lane: Schraudolph approximate exp — one tensor_scalar
        computing int16(round(A*x + B')) written through an int16 view of
        the bf16 p-tile; the int16 bit pattern IS bf16 2^(x*log2e).
        (1024+120)cyc @0.96GHz = 1192ns/block. ~40% of blocks; rel err
        contribution ~±3% on P, ~9e-3 on output (validated numerically).
  - AV transposed: per q-block, matmul(o[128q, 65], lhsT=P_block[128k,128q]
    (weights), rhs=V'[128k,65]) -> 65 moving cols x 8 q-blocks = 520
    cyc/block (vs 1024 in v1), and the output lands in [q, d] layout.
    V' has a ones column so col 64 of o accumulates the softmax denominator
    per PARTITION (q) — reciprocal + stride-0-broadcast tensor_tensor
    normalize replace v1's broadcast-DMA epilogue entirely.
  - PE becomes the bottleneck: 43 blocks x (1024+520)cyc @2.4GHz = 27.7us.
    PE stream is kept dense (QK(i+1) ... AV(i-2), software pipeline lag 2)
    because any PE gap resets the clock ramp to 1.2GHz.
  - Warm matmuls on a memset tile start the PE clock ramp at ~0.4us so
    real QKs run at full speed from ~3.4us.
"""

import math
from contextlib import ExitStack

import numpy as np
import ml_dtypes

import concourse.bass as bass
import concourse.bacc as bacc
import concourse.mybir as mybir
import concourse.tile as tile
from concourse.bass_utils import run_bass_kernel_spmd

F32 = mybir.dt.float32
BF16 = mybir.dt.bfloat16
I16 = mybir.dt.int16
EXP = mybir.ActivationFunctionType.Exp
MULT = None  # set below

B, L, D = 64, 1024, 64
N_CORES = 8
SLOTS = B // N_CORES
KB = 128
N_KB = L // KB
QH = 512
NEG_B = -20.0          # mask bias in exp-argument units (exp(-20+5.5)≈5e-7)
A16 = 2.0**7 / math.log(2.0)
B16 = 127.0 * 2.0**7
C16 = 4.0              # Schraudolph centering constant (round-to-nearest)

_SLOT_ORDERS = {
    "A": [2, 3, 4, 5, 6, 7, 0, 1],
    "B": [2, 5, 3, 6, 4, 7, 0, 1],
    "C": [0, 2, 4, 6, 1, 3, 5, 7][::-1],
    "D": [3, 2, 5, 4, 7, 6, 1, 0],
}
SLOT_ORDER = None  # set in build via SLOT_ORDER_ID
PAIR_ORDER = [1, 2, 3, 0]
DVE_FRAC = 17.0 / 43.0
COPY_MODE = "act"
AV_LAG = 3
FIRST_SWDGE = False
SPLIT_KB0 = False
SLOT_ORDER_ID = "A"
N_WARM = 5
DVE_SET_ID = "S7"
DVE_SETS = {
    "S1": {(2,1),(2,3),(2,5),(3,1),(3,3),(3,5),(4,1),(4,3),(5,1),(5,3),
           (6,2),(7,1),(0,1),(0,3),(0,5),(0,7),(1,1),(1,3),(1,5)},
    # boundary-last blocks back to ACT
    "S2": {(2,1),(2,3),(2,5),(3,1),(3,3),(4,1),(4,3),(5,1),(5,3),
           (6,1),(7,1),(0,1),(0,3),(0,5),(1,1),(1,3),(1,5)},
    # n=21: denser
    "S3": {(2,1),(2,3),(2,5),(3,1),(3,3),(3,5),(4,1),(4,3),(5,1),(5,3),
           (6,1),(7,1),(0,1),(0,3),(0,5),(0,7),(1,1),(1,3),(1,5),(1,7),(4,2)},
    # n=15: lighter DVE
    "S4": {(2,1),(2,3),(3,1),(3,3),(4,1),(4,3),(5,1),(5,3),
           (6,2),(7,1),(0,1),(0,3),(0,5),(1,1),(1,3)},
    # n=23: S3 + 2 more
    "S5": {(2,1),(2,3),(2,5),(3,1),(3,3),(3,5),(4,1),(4,2),(4,3),(5,1),(5,3),
           (6,1),(7,1),(0,1),(0,3),(0,5),(0,7),(1,1),(1,3),(1,5),(1,7),(2,4),(3,4)},
    # n=25
    "S6": {(2,1),(2,3),(2,4),(2,5),(3,1),(3,3),(3,4),(3,5),(4,1),(4,2),(4,3),
           (5,1),(5,2),(5,3),(6,1),(7,1),(0,1),(0,3),(0,5),(0,6),(0,7),
           (1,1),(1,3),(1,5),(1,7)},
    # n=21 like S3 but different short-slot choices
    "S7": {(2,1),(2,3),(2,5),(3,1),(3,3),(3,5),(4,1),(4,3),(5,1),(5,3),
           (6,1),(6,2),(7,1),(0,1),(0,3),(0,5),(0,7),(1,1),(1,3),(1,5),(1,7)},
}


def build_kernel(counts):
    global SLOT_ORDER
    SLOT_ORDER = _SLOT_ORDERS[SLOT_ORDER_ID]
    nc = bacc.Bacc()
    mult = mybir.AluOpType.mult
    add = mybir.AluOpType.add

    qt_d = nc.dram_tensor("qt", [SLOTS // 2, 2 * D, L], BF16, kind="ExternalInput")
    kt_d = nc.dram_tensor("kt", [SLOTS // 2, 2 * D, L], BF16, kind="ExternalInput")
    v_d = nc.dram_tensor("v", [SLOTS, KB, N_KB, D + 1], BF16, kind="ExternalInput")
    bias_d = nc.dram_tensor("bias", [KB, SLOTS * N_KB], F32, kind="ExternalInput")
    cadd_d = nc.dram_tensor("cadd", [KB, SLOTS * N_KB], F32, kind="ExternalInput")
    out_d = nc.dram_tensor("out", [SLOTS, KB, N_KB, D], BF16, kind="ExternalOutput")
    DEBUG = False
    if DEBUG:
        pdump_d = nc.dram_tensor("pdump", [64, KB, L], BF16, kind="ExternalOutput")
        sdump_d = nc.dram_tensor("sdump", [64, KB, L], F32, kind="ExternalOutput")
        odump_d = nc.dram_tensor("odump", [SLOTS, KB, N_KB * (D + 1)], F32, kind="ExternalOutput")

    work = [(s, kb) for s in SLOT_ORDER for kb in range(counts[s])]
    n_work = len(work)
    # Lane assignment, tuned numerically for this counts profile: DVE
    # (approximate exp) on odd / masked-tail key-blocks (lowest softmax
    # weight), never consecutive, ACT runs <= 3 so neither engine stalls
    # the PE. Rel-err with C16=5.5: 7.4e-3 (budget 2e-2).
    DVE_SET = DVE_SETS[DVE_SET_ID]
    if DVE_FRAC == 0.0:
        DVE_SET = set()
    lanes = ["dve" if wk in DVE_SET else "act" for wk in work]

    next_pair = {PAIR_ORDER[i]: PAIR_ORDER[i + 1] for i in range(len(PAIR_ORDER) - 1)}
    slot_last = {}
    for i, (s, kb) in enumerate(work):
        slot_last[s] = i

    with tile.TileContext(nc) as tc, ExitStack() as ctx:
        const_pool = ctx.enter_context(tc.tile_pool(name="const", bufs=1))
        qk_pool = ctx.enter_context(tc.tile_pool(name="qk", bufs=3))
        v_pool = ctx.enter_context(tc.tile_pool(name="v", bufs=4))
        # Deep p pool: the exp lanes run ~1 block of latency behind the PE;
        # 10 bufs keeps exp(i+k) from blocking on AV(i) reads (2.2us win).
        p_pool = ctx.enter_context(tc.tile_pool(name="p", bufs=10))
        y_pool = ctx.enter_context(tc.tile_pool(name="y", bufs=3))
        osb_pool = ctx.enter_context(tc.tile_pool(name="osb", bufs=2))
        rec_pool = ctx.enter_context(tc.tile_pool(name="rec", bufs=2))
        psum_s = ctx.enter_context(tc.tile_pool(name="psum_s", bufs=3, space="PSUM"))
        psum_o = ctx.enter_context(tc.tile_pool(name="psum_o", bufs=1, space="PSUM"))

        bias_t = const_pool.tile([KB, SLOTS * N_KB], F32)
        cadd_t = const_pool.tile([KB, SLOTS * N_KB], F32)
        warm_t = const_pool.tile([D, QH], BF16)
        wact_t = const_pool.tile([1, 1], F32)

        # Warm-up: memset source tile, start ACT table load, start PE ramp.
        nc.gpsimd.memset(warm_t[:], 0.0)
        nc.gpsimd.memset(wact_t[:], 0.0)
        nc.scalar.activation(wact_t[:], wact_t[:], EXP)

        pair_tiles: dict[int, tuple] = {}
        v_tiles: dict[int, object] = {}

        def load_pair(p, first=False):
            if p in pair_tiles:
                return
            n_max = counts[2 * p]
            qt_t = qk_pool.tile([2 * D, L], BF16, tag="qt", name="qt_t")
            kt_t = qk_pool.tile([2 * D, L], BF16, tag="kt", name="kt_t")
            if first:
                # kt head on sync, first q-half on vector: first QK waits
                # on ~160KB across two queues.
                nc.sync.dma_start(kt_t[:, :KB], kt_d[p][:, :KB])
                nc.scalar.dma_start(qt_t[:, :QH], qt_d[p][:, :QH])
                if FIRST_SWDGE:
                    nc.gpsimd.dma_start(qt_t[:, QH:], qt_d[p][:, QH:])
                else:
                    nc.sync.dma_start(qt_t[:, QH:], qt_d[p][:, QH:])
                if n_max > 1:
                    # kb1's keys ride their own small DMA so block 1's QK
                    # is not gated on the whole kt tail transfer.
                    nc.sync.dma_start(kt_t[:, KB : 2 * KB], kt_d[p][:, KB : 2 * KB])
                    nc.sync.dma_start(
                        kt_t[:, 2 * KB : n_max * KB], kt_d[p][:, 2 * KB : n_max * KB]
                    )
            else:
                nc.sync.dma_start(qt_t[:], qt_d[p][:])
                nc.sync.dma_start(kt_t[:, : n_max * KB], kt_d[p][:, : n_max * KB])
            pair_tiles[p] = (qt_t, kt_t)

        def load_v(s):
            if s in v_tiles:
                return
            v_t = v_pool.tile([KB, N_KB, D + 1], BF16, name="v_t")
            nc.gpsimd.dma_start(v_t[:], v_d[s][:])
            v_tiles[s] = v_t

        load_pair(PAIR_ORDER[0], first=True)
        nc.gpsimd.dma_start(bias_t[:], bias_d[:])
        nc.gpsimd.dma_start(cadd_t[:], cadd_d[:])
        load_v(SLOT_ORDER[0])
        load_v(SLOT_ORDER[1])

        # PE clock-ramp warm matmuls (data-independent).
        for w in range(N_WARM):
            w_ps = psum_s.tile([KB, L], F32, tag="s", name="warm_ps")
            nc.tensor.matmul(
                w_ps[:D, :QH], warm_t[:, :D], warm_t[:, :QH],
                start=True, stop=True,
            )

        s_tiles: dict[int, object] = {}
        p_tiles: dict[int, object] = {}
        o_tile = [None]
        pending_out: list = []

        def emit_qk(i):
            s, kb = work[i]
            pair, half = divmod(s, 2)
            if kb == 0:
                idx = SLOT_ORDER.index(s)
                if idx + 2 < SLOTS:
                    load_v(SLOT_ORDER[idx + 2])
                if half == 0 and pair in next_pair:
                    load_pair(next_pair[pair])
            qt_t, kt_t = pair_tiles[pair]
            rows = slice(D * half, D * half + D)
            s_t = psum_s.tile([KB, L], F32, tag="s", name="s_ps")
            s_tiles[i] = s_t
            for qh in range(2):
                nc.tensor.matmul(
                    s_t[:, qh * QH : (qh + 1) * QH],
                    kt_t[rows, kb * KB : (kb + 1) * KB],
                    qt_t[rows, qh * QH : (qh + 1) * QH],
                    start=True,
                    stop=True,
                )

        def emit_exp(i):
            s, kb = work[i]
            col = s * N_KB + kb
            p_t = p_pool.tile([KB, L], BF16, name="p_t")
            p_tiles[i] = p_t
            s_t = s_tiles.pop(i)
            if SPLIT_KB0 and kb == 0 and s != SLOT_ORDER[0] and lanes[i] == "act":
                # boundary block: split exp across both lanes
                nc.scalar.activation(
                    p_t[:, :QH], s_t[:, :QH], EXP,
                    bias=bias_t[:, col : col + 1],
                    scale=1.0 / math.sqrt(D),
                )
                nc.vector.tensor_scalar(
                    p_t[:, QH:].bitcast(I16), s_t[:, QH:],
                    A16 / math.sqrt(D),
                    cadd_t[:, col : col + 1],
                    op0=mult, op1=add,
                )
            elif lanes[i] == "act":
                nc.scalar.activation(
                    p_t[:], s_t[:], EXP,
                    bias=bias_t[:, col : col + 1],
                    scale=1.0 / math.sqrt(D),
                )
            else:
                nc.vector.tensor_scalar(
                    p_t[:].bitcast(I16), s_t[:],
                    A16 / math.sqrt(D),
                    cadd_t[:, col : col + 1],
                    op0=mult, op1=add,
                )
            if DEBUG:
                dtmp = p_pool.tile([KB, L], F32, tag="dbg", name="dbg_t")
                nc.vector.tensor_copy(dtmp[:], s_t[:])
                nc.scalar.dma_start(sdump_d[i][:], dtmp[:])
                nc.scalar.dma_start(pdump_d[i][:], p_t[:])

        def emit_av(i):
            s, kb = work[i]
            n_kb = counts[s]
            if kb == 2 and pending_out:
                ps, py = pending_out.pop(0)
                nc.sync.dma_start(out_d[ps][:], py[:])
            if kb == 0:
                o_tile[0] = psum_o.tile([KB, N_KB * (D + 1)], F32, name="o_ps")
            o_ps = o_tile[0]
            p_t = p_tiles.pop(i)
            if kb == 0:
                # PSUM start=True zeroing is only reliable when the zeroing
                # matmul covers the region itself: zero the whole o tile
                # with two dummy matmuls from the (zero) warm tile, then
                # every real AV accumulates with start=False.
                nc.tensor.matmul(
                    o_ps[:, :QH], warm_t[:, :KB], warm_t[:, :QH],
                    start=True, stop=False, skip_group_check=True,
                )
                nc.tensor.matmul(
                    o_ps[:, QH : N_KB * (D + 1)],
                    warm_t[:, :KB], warm_t[:, : N_KB * (D + 1) - QH],
                    start=True, stop=False, skip_group_check=True,
                )
            for qb in range(N_KB):
                nc.tensor.matmul(
                    o_ps[:, qb * (D + 1) : (qb + 1) * (D + 1)],
                    p_t[:, qb * KB : (qb + 1) * KB],
                    v_tiles[s][:, kb, :],
                    start=False,
                    stop=(kb == n_kb - 1 and qb == N_KB - 1),
                    skip_group_check=True,
                )
            if i == slot_last[s]:
                emit_epilogue(s, o_ps)

        def emit_epilogue(s, o_ps):
            if DEBUG:
                otmp = p_pool.tile([KB, N_KB * (D + 1)], F32, tag="odbg", name="odbg_t")
                nc.vector.tensor_copy(otmp[:], o_ps[:])
                nc.scalar.dma_start(odump_d[s][:], otmp[:])
            # Epilogue spread across engines so no lane takes a long
            # excursion at slot boundaries: DVE reciprocal (tiny, PSUM),
            # ACT/DVE alternate evacuating o to SBUF, Pool (otherwise
            # idle) normalizes.
            rec_t = rec_pool.tile([KB, N_KB], F32, name="rec_t")
            o_ap = o_ps[:]
            z_view = bass.AP(
                o_ap.tensor, o_ap.offset + D,
                [list(o_ap.ap)[0], [D + 1, N_KB]],
            )
            nc.vector.reciprocal(rec_t[:], z_view)
            r_ap = rec_t[:]
            if s == SLOT_ORDER[-1]:
                # Tail: normalize straight from PSUM on DVE in two halves
                # and ship each half on its own DMA queue so the final
                # DMA latency pipeline starts as early as possible.
                H = N_KB // 2
                for h, queue in ((0, nc.sync), (1, nc.scalar)):
                    y_t = y_pool.tile([KB, H, D], BF16, name="y_tail")
                    o_view = bass.AP(
                        o_ap.tensor, o_ap.offset + h * H * (D + 1),
                        [list(o_ap.ap)[0], [D + 1, H], [1, D]],
                    )
                    r_bcast = bass.AP(
                        r_ap.tensor, r_ap.offset + h * H,
                        [list(r_ap.ap)[0], [1, H], [0, D]],
                    )
                    nc.vector.tensor_tensor(y_t[:], o_view, r_bcast, op=mult)
                    queue.dma_start(out_d[s][:, h * H : (h + 1) * H, :], y_t[:])
                return
            osb_t = osb_pool.tile([KB, N_KB * (D + 1)], F32, name="osb_t")
            if COPY_MODE == "dve":
                nc.vector.tensor_copy(osb_t[:], o_ps[:])
            elif COPY_MODE == "act":
                nc.scalar.copy(osb_t[:], o_ps[:])
            elif COPY_MODE == "alt" and SLOT_ORDER.index(s) % 2 == 1:
                nc.vector.tensor_copy(osb_t[:], o_ps[:])
            elif COPY_MODE == "alt":
                nc.scalar.copy(osb_t[:], o_ps[:])
            else:
                HW_ = (N_KB // 2) * (D + 1)
                nc.scalar.copy(osb_t[:, :HW_], o_ps[:, :HW_])
                nc.vector.tensor_copy(osb_t[:, HW_:], o_ps[:, HW_:])
            y_t = y_pool.tile([KB, N_KB, D], BF16, name="y_t")
            osb_ap = osb_t[:]
            o_view = bass.AP(
                osb_ap.tensor, osb_ap.offset,
                [list(osb_ap.ap)[0], [D + 1, N_KB], [1, D]],
            )
            r_bcast = bass.AP(
                r_ap.tensor, r_ap.offset,
                [list(r_ap.ap)[0], [1, N_KB], [0, D]],
            )
            nc.gpsimd.tensor_tensor(y_t[:], o_view, r_bcast, op=mult)
            # Defer the out DMA: issuing it now would block SP.SEQ (and the
            # input prefetches queued behind it) until the norm completes.
            pending_out.append((s, y_t))

        for j in range(AV_LAG):
            emit_qk(j)
        for i in range(n_work):
            if i >= AV_LAG:
                emit_av(i - AV_LAG)
            emit_exp(i)
            if i + AV_LAG < n_work:
                emit_qk(i + AV_LAG)
        for j in range(AV_LAG):
            emit_av(n_work - AV_LAG + j)
        for ps, py in pending_out:
            nc.sync.dma_start(out_d[ps][:], py[:])

    nc.finalize()
    return nc


_NC_CACHE: dict[tuple, object] = {}


def _prepare(queries, keys, values, valid_lens):
    queries = np.ascontiguousarray(queries, dtype=np.float32)
    keys = np.ascontiguousarray(keys, dtype=np.float32)
    values = np.ascontiguousarray(values, dtype=np.float32)
    vl = np.asarray(valid_lens).astype(np.int64)
    assert queries.shape == (B, L, D), queries.shape

    order = np.argsort(-vl, kind="stable")
    counts = tuple(
        max(1, math.ceil(int(vl[order[s * N_CORES]]) / KB)) for s in range(SLOTS)
    )
    nc = _NC_CACHE.get(counts)
    if nc is None:
        nc = build_kernel(counts)
        _NC_CACHE[counts] = nc

    col = np.arange(L)
    bf = ml_dtypes.bfloat16
    in_maps = []
    for c in range(N_CORES):
        batch_idx = [int(order[s * N_CORES + c]) for s in range(SLOTS)]
        # qt/kt pairs: [4, 128, 1024] = two batches' [D, L] stacked
        qt = (
            queries[batch_idx].transpose(0, 2, 1).reshape(SLOTS // 2, 2 * D, L)
        ).astype(bf)
        kt = (
            keys[batch_idx].transpose(0, 2, 1).reshape(SLOTS // 2, 2 * D, L)
        ).astype(bf)
        # v: [8, 128, 8, 65] p-major with ones column
        vv = values[batch_idx].reshape(SLOTS, N_KB, KB, D).transpose(0, 2, 1, 3)
        v = np.concatenate(
            [vv, np.ones((SLOTS, KB, N_KB, 1), np.float32)], axis=3
        ).astype(bf)
        bias = np.zeros((KB, SLOTS * N_KB), dtype=np.float32)
        for s in range(SLOTS):
            mask = (col >= vl[batch_idx[s]]).astype(np.float32) * NEG_B  # [L]
            bias[:, s * N_KB : (s + 1) * N_KB] = mask.reshape(N_KB, KB).T
        cadd = (A16 * bias + (B16 - C16)).astype(np.float32)
        in_maps.append(
            {
                "qt": qt.view(np.uint16),
                "kt": kt.view(np.uint16),
                "v": v.view(np.uint16),
                "bias": bias,
                "cadd": cadd,
            }
        )
    return nc, in_maps, order


def _unshard(res, order):
    bf = ml_dtypes.bfloat16
    out = np.empty((B, L, D), dtype=np.float32)
    for c in range(N_CORES):
        o = np.asarray(res.results[c]["out"])
        if o.dtype != bf:
            o = o.view(bf)
        o = o.astype(np.float32)  # [SLOTS, 128, 8, 64]
        for s in range(SLOTS):
            out[int(order[s * N_CORES + c])] = (
                o[s].transpose(1, 0, 2).reshape(L, D)
            )
    return out


def kernel(queries, keys, values, valid_lens):
    nc, in_maps, order = _prepare(queries, keys, values, valid_lens)
    res = run_bass_kernel_spmd(nc, in_maps, core_ids=list(range(N_CORES)))
    return _unshard(res, order)


def trace_run(queries, keys, values, valid_lens):
    nc, in_maps, order = _prepare(queries, keys, values, valid_lens)
    res = run_bass_kernel_spmd(
        nc, in_maps, core_ids=list(range(N_CORES)), trace=True
    )
    res.full_output = _unshard(res, order)
    return res

